# revision 4
# baseline (speedup 1.0000x reference)
"""Trainium2 Bass kernel for nn_DecoderLayer_73280732004792.

Decoder layer: self-attn (image weights) + cross-attn (image Q / audio KV)
+ FFN, three layernorms. B=4, S=1024, D=1024, H=16, HD=64, FF=4096, fp32 IO.

Sharding: 8 cores = (batch, seq-half). Each core owns 512 query rows and
recomputes K/V projections over its batch's full 1024-row sequence locally,
so there are no collectives. Activations live feature-major in SBUF
([128 feat-partitions, ktile, tokens]); the host pre-transposes weights and
activations so every DMA is contiguous. Matmuls run in bf16 with fp32 PSUM
accumulation; layernorm statistics use f32r (full-rate fp32) matmuls against
a 1/D column. Softmax is computed along the partition axis: scores come out
of the PE transposed [j, i], exp runs on the scalar engine straight out of
PSUM, the denominator is picked up by a ones-column appended to V (M=65 AV
matmul), and the reciprocal is broadcast back across partitions with a K=1
matmul. Max-subtraction is skipped: logits have std ~0.4 here, |s| < ~15.
"""

import os
import sys

import numpy as np


def _ensure_path():
    try:
        import concourse  # noqa: F401

        return
    except ImportError:
        pass
    for p in ("/opt/trn_rl_repo", os.path.expanduser("~/.axon_site/_ro/trn_rl_repo")):
        if os.path.isdir(p) and p not in sys.path:
            sys.path.insert(0, p)


_ensure_path()

import ml_dtypes  # noqa: E402

import concourse.bass as bass  # noqa: E402
import concourse.mybir as mybir  # noqa: E402
import concourse.tile as tile  # noqa: E402
from concourse import bacc  # noqa: E402

B, S, D, H, HD, FF = 4, 1024, 1024, 16, 64, 4096
P = 128
T = 512  # tokens owned per core
KT = D // P  # 8 k-tiles over the model dim
N_CORES = 8

F32 = mybir.dt.float32
F32R = mybir.dt.float32r
BF16 = mybir.dt.bfloat16
AF = mybir.ActivationFunctionType
OP = mybir.AluOpType

# bias_pack column groups (8 cols each, feature-major [128, 8])
BQ, BKI, BKA, G1, B1C, G2, B2, G3, B3, FB2, SOB = range(11)
FB1_COL = 88  # ffn_b1 occupies cols 88..119
BIAS_COLS = 120

_NC = None  # cached compiled program


def _build_program():
    nc = bacc.Bacc(None, target_bir_lowering=False, debug=False)

    def din(name, shape, dt=BF16):
        return nc.dram_tensor(name, list(shape), dt, kind="ExternalInput").ap()

    # Per-core activations (host pre-transposed, feature-major)
    xTi = din("xTi", [D, S])           # image_outputs[b].T, own 512 tokens first
    xTx = din("xTx", [D, S])           # x[b].T
    cyT = din("cyT", [D, T])           # constant_y[b, rows].T
    yT = din("yT", [D, T], F32R)       # y[b, rows].T (f32r so it can feed LN-stat matmuls)
    # Weights (all pre-transposed to [in_features, out_features])
    wqi = din("wqi", [D, D])
    wki = din("wki", [D, D])
    wvi = din("wvi", [D, D])
    wka = din("wka", [D, D])
    wva = din("wva", [D, D])
    wso = din("wso", [D, D])
    wco = din("wco", [D, D])
    w1 = din("w1", [D, FF])
    w2 = din("w2", [FF, D])
    bias_pack = din("bias_pack", [P, BIAS_COLS], F32)
    constA = din("constA", [P, 1], F32R)   # 1/D column
    constB = din("constB", [1, P], F32R)   # ones row
    vb = din("vb", [P, 2 * D], F32)    # value biases broadcast over partitions
    y3T = nc.dram_tensor("y3T", [D, T], F32, kind="ExternalOutput").ap()

    def re3(ap):  # [D, t] dram -> [128, D//128, t]
        return ap.rearrange("(o p) t -> p o t", p=P)

    xTi_r, xTx_r, cyT_r, yT_r = re3(xTi), re3(xTx), re3(cyT), re3(yT)
    wqi_r, wki_r, wvi_r = re3(wqi), re3(wki), re3(wvi)
    wka_r, wva_r, wso_r, wco_r = re3(wka), re3(wva), re3(wso), re3(wco)
    w1_r = re3(w1)          # [128, 8, 4096]
    w2_r = re3(w2)          # [128, 32, 1024]
    y3_r = re3(y3T)

    with tile.TileContext(nc) as tc, nc.allow_low_precision(
        reason="float32r tiles carry full fp32 bits; PSUM accumulation stays fp32"
    ):
        with (
            tc.tile_pool(name="const", bufs=1) as pc,
            tc.tile_pool(name="resid", bufs=1) as pres,
            tc.tile_pool(name="small", bufs=6) as psm,
            tc.tile_pool(name="expp", bufs=3) as pexp,
            tc.tile_pool(name="psum", bufs=1, space="PSUM") as pp,
        ):
            bias_sb = pc.tile([P, BIAS_COLS], F32)
            nc.sync.dma_start(bias_sb[:], bias_pack[:])
            vb_sb = pc.tile([P, 2 * D], F32)
            nc.sync.dma_start(vb_sb[:], vb[:])
            onesD = pc.tile([P, 1], F32R)
            nc.sync.dma_start(onesD[:], constA[:])
            ones1 = pc.tile([1, P], F32R)
            nc.sync.dma_start(ones1[:], constB[:])
            eps_sb = pc.tile([1, 1], F32)
            nc.vector.memset(eps_sb[:], 1e-5)

            y2 = pres.tile([P, KT, T], F32, tag="y2")

            def bcol(grp, kt):
                return bias_sb[:, grp * 8 + kt : grp * 8 + kt + 1]

            def ws_proj(w_re, x_sb, ntok, evict, nof=D):
                """Weight-stationary GEMM: out^T[of, t] = sum_k W^T[k, of] x^T[k, t].

                Yields once per (of-tile, token-block) psum eviction.
                """
                for c0 in range(0, nof, 512):
                    wch = pw.tile([P, KT, 512], BF16, tag="w", bufs=2)
                    nc.sync.dma_start(wch[:], w_re[:, :, c0 : c0 + 512])
                    for lot in range(4):
                        ot = (c0 + lot * P) // P
                        for tb in range(ntok // 512):
                            ps = pp.tile([P, 512], F32, tag="proj", bufs=2)
                            for kt in range(KT):
                                nc.tensor.matmul(
                                    ps[:],
                                    wch[:, kt, lot * P : (lot + 1) * P],
                                    x_sb[:, kt, tb * 512 : (tb + 1) * 512],
                                    start=(kt == 0),
                                    stop=(kt == KT - 1),
                                )
                            evict(ps, ot, tb)
                            yield

            def v_proj(w_re, x_sb, vb_off):
                """Activation-stationary GEMM producing token-major V with a
                ones column at index 64 of each head (softmax denominator)."""
                vA = pa.tile([P, KT, H, HD + 1], BF16, tag="vA", bufs=2)
                nc.vector.memset(vA[:, :, :, HD : HD + 1], 1.0)
                yield vA
                for oh in range(2):
                    wch = pw.tile([P, KT, 512], BF16, tag="w", bufs=2)
                    nc.sync.dma_start(wch[:], w_re[:, :, oh * 512 : (oh + 1) * 512])
                    for tt in range(8):
                        ps = pp.tile([P, 512], F32, tag="proj", bufs=2)
                        for kt in range(KT):
                            nc.tensor.matmul(
                                ps[:],
                                x_sb[:, kt, tt * P : (tt + 1) * P],
                                wch[:, kt, :],
                                start=(kt == 0),
                                stop=(kt == KT - 1),
                            )
                        nc.vector.tensor_tensor(
                            vA[:, tt, oh * 8 : (oh + 1) * 8, 0:HD],
                            ps[:].rearrange("p (a c) -> p a c", a=8),
                            vb_sb[:, vb_off + oh * 512 : vb_off + (oh + 1) * 512].rearrange(
                                "p (a c) -> p a c", a=8
                            ),
                            OP.add,
                        )
                        yield vA

            def attn_head(h, q_t, k_t, vA_t, o_t):
                po, ko = HD * (h % 2), h // 2
                o_ps = pp.tile([HD + 1, 512], F32, tag="av", bufs=1)
                for jt in range(KT):
                    s_ps = pp.tile([P, 512], F32, tag="score", bufs=2)
                    nc.tensor.matmul(
                        s_ps[:],
                        k_t[po : po + HD, ko, jt * P : (jt + 1) * P],
                        q_t[po : po + HD, ko, :],
                        start=True,
                        stop=True,
                    )
                    e = pexp.tile([P, 512], BF16, tag="exp", bufs=3)
                    nc.scalar.activation(e[:], s_ps[:], AF.Exp)
                    nc.tensor.matmul(
                        o_ps[:], vA_t[:, jt, h, :], e[:], start=(jt == 0), stop=(jt == KT - 1)
                    )
                rd = psm.tile([1, 512], F32R, tag="rd", bufs=3)
                nc.vector.reciprocal(rd[:], o_ps[HD : HD + 1, :])
                r_ps = pp.tile([HD, 512], F32, tag="misc", bufs=3)
                nc.tensor.matmul(
                    r_ps[:],
                    ones1[:1, 0:HD],
                    rd[:],
                    start=True,
                    stop=True,
                )
                rb_sb = psm.tile([HD, 512], BF16, tag="rb", bufs=2)
                nc.vector.tensor_copy(rb_sb[:], r_ps[:])
                nc.vector.tensor_tensor(
                    o_t[po : po + HD, ko, :], o_ps[0:HD, :], rb_sb[:], OP.mult
                )

            def layernorm(r_t, out_t, g_grp, b_grp, sq_pool):
                """out = g * (r - mean) / sqrt(var + eps) + b along partitions."""
                m_ps = pp.tile([1, 512], F32, tag="misc", bufs=3)
                for kt in range(KT):
                    nc.tensor.matmul(
                        m_ps[:],
                        onesD[:],
                        r_t[:, kt, :],
                        start=(kt == 0),
                        stop=(kt == KT - 1),
                    )
                yield
                ss_ps = pp.tile([1, 512], F32, tag="misc", bufs=3)
                for kt in range(KT):
                    sq = sq_pool.tile([P, 512], F32R, tag="sq", bufs=2)
                    nc.scalar.activation(sq[:], r_t[:, kt, :], AF.Square)
                    nc.tensor.matmul(
                        ss_ps[:],
                        onesD[:],
                        sq[:],
                        start=(kt == 0),
                        stop=(kt == KT - 1),
                    )
                    if kt % 2 == 1:
                        yield
                m_sb = psm.tile([1, 512], F32R, tag="st", bufs=6)
                nc.vector.tensor_copy(m_sb[:], m_ps[:])
                var = psm.tile([1, 512], F32, tag="st", bufs=6)
                nc.vector.tensor_tensor(var[:], m_sb[:], m_sb[:], OP.mult)
                nc.vector.tensor_tensor(var[:], ss_ps[:], var[:], OP.subtract)
                rstd = psm.tile([1, 512], F32R, tag="st", bufs=6)
                nc.scalar.activation(rstd[:], var[:], AF.Sqrt, bias=eps_sb[:])
                nc.vector.reciprocal(rstd[:], rstd[:])
                mb = pp.tile([P, 512], F32, tag="misc", bufs=3)
                nc.tensor.matmul(
                    mb[:], ones1[:], m_sb[:], start=True, stop=True
                )
                rb = pp.tile([P, 512], F32, tag="misc", bufs=3)
                nc.tensor.matmul(
                    rb[:], ones1[:], rstd[:], start=True, stop=True
                )
                yield
                for kt in range(KT):
                    o = out_t[:, kt, :]
                    nc.vector.tensor_tensor(o, r_t[:, kt, :], mb[:], OP.subtract)
                    nc.vector.tensor_tensor(o, o, rb[:], OP.mult)
                    nc.vector.tensor_scalar(
                        o, o, bcol(g_grp, kt), bcol(b_grp, kt), OP.mult, OP.add
                    )
                    if kt % 3 == 2:
                        yield

            def drain(gen, n):
                for _ in range(n):
                    if next(gen, _SENT) is _SENT:
                        return True
                return False

            _SENT = object()

            with (
                tc.tile_pool(name="attn", bufs=1) as pa,
                tc.tile_pool(name="wts", bufs=2) as pw,
            ):
                # ---- self-attention QKV (image weights on image_outputs) ----
                xTi_sb = pa.tile([P, KT, S], BF16, tag="xT", bufs=1)
                nc.sync.dma_start(xTi_sb[:], xTi_r)
                rT1 = pa.tile([P, KT, T], F32R, tag="rT", bufs=1)
                nc.sync.dma_start(rT1[:], yT_r)
                for kt in range(KT):  # fold self_out_b into the residual
                    nc.vector.tensor_scalar_add(rT1[:, kt, :], rT1[:, kt, :], bcol(SOB, kt))

                q1 = pa.tile([P, KT, T], BF16, tag="q", bufs=2)

                def q1_evict(ps, ot, tb):
                    nc.vector.tensor_scalar(
                        q1[:, ot, :], ps[:], bcol(BQ, ot), 1.0 / 8.0, OP.add, OP.mult
                    )

                k1 = pa.tile([P, KT, S], BF16, tag="k", bufs=1)

                def k1_evict(ps, ot, tb):
                    nc.vector.tensor_scalar_add(
                        k1[:, ot, tb * 512 : (tb + 1) * 512], ps[:], bcol(BKI, ot)
                    )

                for _ in ws_proj(wqi_r, xTi_sb, T, q1_evict):
                    pass
                for _ in ws_proj(wki_r, xTi_sb, S, k1_evict):
                    pass
                gv1 = v_proj(wvi_r, xTi_sb, 0)
                vA1 = next(gv1)
                for _ in gv1:
                    pass

                # ---- self attention, interleaved with cross Q and cross V ----
                o1 = pa.tile([P, KT, T], BF16, tag="o", bufs=2)

                q2 = pa.tile([P, KT, T], BF16, tag="q", bufs=2)

                def q2_evict(ps, ot, tb):
                    nc.vector.tensor_scalar(
                        q2[:, ot, :], ps[:], bcol(BQ, ot), 1.0 / 8.0, OP.add, OP.mult
                    )

                cross_t = {}

                def cross_fillers():
                    cyT_sb = pa.tile([P, KT, T], BF16, tag="cyT", bufs=1)
                    nc.sync.dma_start(cyT_sb[:], cyT_r)
                    yield
                    for u in ws_proj(wqi_r, cyT_sb, T, q2_evict):
                        yield
                    xTx_sb = pa.tile([P, KT, S], BF16, tag="xT", bufs=1)
                    nc.sync.dma_start(xTx_sb[:], xTx_r)
                    cross_t["xTx"] = xTx_sb
                    yield
                    gv2 = v_proj(wva_r, xTx_sb, D)
                    cross_t["vA2"] = next(gv2)
                    for _ in gv2:
                        yield

                fill1 = cross_fillers()
                for h in range(H):
                    attn_head(h, q1, k1, vA1, o1)
                    drain(fill1, 2)
                for _ in fill1:
                    pass
                xTx_sb, vA2 = cross_t["xTx"], cross_t["vA2"]

                # ---- cross K (pure PE stretch) ----
                k2 = pa.tile([P, KT, S], BF16, tag="k", bufs=1)

                def k2_evict(ps, ot, tb):
                    nc.vector.tensor_scalar_add(
                        k2[:, ot, tb * 512 : (tb + 1) * 512], ps[:], bcol(BKA, ot)
                    )

                for _ in ws_proj(wka_r, xTx_sb, S, k2_evict):
                    pass

                # ---- cross attention, interleaved with self out-proj + LN1 ----
                o2 = pa.tile([P, KT, T], BF16, tag="o", bufs=2)
                y1 = pa.tile([P, KT, T], F32, tag="y1", bufs=1)

                def so_evict(ps, ot, tb):
                    nc.vector.tensor_tensor(rT1[:, ot, :], ps[:], rT1[:, ot, :], OP.add)

                def fillers2():
                    for u in ws_proj(wso_r, o1, T, so_evict):
                        yield
                    for u in layernorm(rT1, y1, G1, B1C, pa):
                        yield

                fill2 = fillers2()
                for h in range(H):
                    attn_head(h, q2, k2, vA2, o2)
                    drain(fill2, 2)
                for _ in fill2:
                    pass

                # ---- cross out-proj + LN2 ----
                rT2 = pa.tile([P, KT, T], F32R, tag="rT", bufs=1)

                def co_evict(ps, ot, tb):
                    nc.vector.tensor_tensor(rT2[:, ot, :], ps[:], y1[:, ot, :], OP.add)

                for _ in ws_proj(wco_r, o2, T, co_evict):
                    pass
                for _ in layernorm(rT2, y2, G2, B2, pa):
                    pass

            # ---- FFN ----
            with (
                tc.tile_pool(name="ffn", bufs=1) as pf,
                tc.tile_pool(name="wts2", bufs=5) as pwf,
            ):
                y2b = pf.tile([P, KT, T], BF16, tag="y2b", bufs=1)
                for kt in range(KT):
                    nc.vector.tensor_copy(y2b[:, kt, :], y2[:, kt, :])

                hT = pf.tile([P, FF // P, T], BF16, tag="hT", bufs=1)
                for oc in range(FF // 512):
                    wch = pwf.tile([P, KT, 512], BF16, tag="w", bufs=5)
                    nc.sync.dma_start(wch[:], w1_r[:, :, oc * 512 : (oc + 1) * 512])
                    for lot in range(4):
                        ot = oc * 4 + lot
                        ps = pp.tile([P, 512], F32, tag="proj", bufs=2)
                        for kt in range(KT):
                            nc.tensor.matmul(
                                ps[:],
                                wch[:, kt, lot * P : (lot + 1) * P],
                                y2b[:, kt, :],
                                start=(kt == 0),
                                stop=(kt == KT - 1),
                            )
                        nc.scalar.activation(
                            hT[:, ot, :],
                            ps[:],
                            AF.Relu,
                            bias=bias_sb[:, FB1_COL + ot : FB1_COL + ot + 1],
                        )

                rT3 = pf.tile([P, KT, T], F32R, tag="rT3", bufs=1)
                for oh in range(2):
                    chs = []
                    for kc in range(4):
                        wch = pwf.tile([P, KT, 512], BF16, tag="w", bufs=5)
                        nc.sync.dma_start(
                            wch[:], w2_r[:, kc * 8 : (kc + 1) * 8, oh * 512 : (oh + 1) * 512]
                        )
                        chs.append(wch)
                    for lot in range(4):
                        ot = oh * 4 + lot
                        ps = pp.tile([P, 512], F32, tag="proj", bufs=2)
                        for kc in range(4):
                            for ko in range(8):
                                kt = kc * 8 + ko
                                nc.tensor.matmul(
                                    ps[:],
                                    chs[kc][:, ko, lot * P : (lot + 1) * P],
                                    hT[:, kt, :],
                                    start=(kt == 0),
                                    stop=(kt == 31),
                                )
                        nc.vector.tensor_scalar_add(rT3[:, ot, :], ps[:], bcol(FB2, ot))
                        nc.vector.tensor_tensor(
                            rT3[:, ot, :], rT3[:, ot, :], y2[:, ot, :], OP.add
                        )

                y3 = pf.tile([P, KT, T], F32, tag="y3", bufs=1)
                for _ in layernorm(rT3, y3, G3, B3, pf):
                    pass
                nc.sync.dma_start(y3_r, y3[:])

    nc.compile()
    return nc


def get_nc():
    global _NC
    if _NC is None:
        _NC = _build_program()
    return _NC


def make_in_maps(inputs):
    """Host-side sharding: per-core input dicts (numpy, pre-transposed)."""
    bf = ml_dtypes.bfloat16
    f32 = np.float32

    def cT(a, dt=bf):  # contiguous transpose + cast
        return np.ascontiguousarray(np.asarray(a, f32).T).astype(dt)

    w_shared = {
        "wqi": cT(inputs["image_query_weight"]),
        "wki": cT(inputs["image_key_weight"]),
        "wvi": cT(inputs["image_value_weight"]),
        "wka": cT(inputs["audio_key_weight"]),
        "wva": cT(inputs["audio_value_weight"]),
        "wso": cT(inputs["self_out_w"]),
        "wco": cT(inputs["cross_out_w"]),
        "w1": cT(inputs["ffn_w1"]),
        "w2": cT(inputs["ffn_w2"]),
    }

    def fm(v):  # [D] -> feature-major [128, KO]
        v = np.asarray(v, f32)
        return v.reshape(-1, P).T

    bias_pack = np.concatenate(
        [
            fm(inputs["image_query_bias"]),
            fm(inputs["image_key_bias"]),
            fm(inputs["audio_key_bias"]),
            fm(inputs["gamma1"]),
            fm(np.asarray(inputs["beta1"], f32) + np.asarray(inputs["cross_out_b"], f32)),
            fm(inputs["gamma2"]),
            fm(inputs["beta2"]),
            fm(inputs["gamma3"]),
            fm(inputs["beta3"]),
            fm(inputs["ffn_b2"]),
            fm(inputs["self_out_b"]),
            fm(inputs["ffn_b1"]),
        ],
        axis=1,
    ).astype(f32)
    assert bias_pack.shape == (P, BIAS_COLS)
    vb = np.concatenate(
        [
            np.broadcast_to(np.asarray(inputs["image_value_bias"], f32), (P, D)),
            np.broadcast_to(np.asarray(inputs["audio_value_bias"], f32), (P, D)),
        ],
        axis=1,
    ).astype(f32)
    w_shared["bias_pack"] = bias_pack
    w_shared["vb"] = vb
    w_shared["constA"] = np.full((P, 1), 1.0 / D, f32)
    w_shared["constB"] = np.ones((1, P), f32)

    x = np.asarray(inputs["x"], f32)
    y = np.asarray(inputs["y"], f32)
    img = np.asarray(inputs["image_outputs"], f32)
    cy = np.asarray(inputs["constant_y"], f32)

    in_maps = []
    for c in range(N_CORES):
        b, half = divmod(c, 2)
        rows = slice(half * T, (half + 1) * T)
        other = slice((1 - half) * T, (2 - half) * T)
        imT = img[b].T  # [D, S]
        # own tokens first so the program's fixed q-slice [0:T] is core-local
        xTi = np.ascontiguousarray(np.concatenate([imT[:, rows], imT[:, other]], 1))
        in_maps.append(
            {
                "xTi": xTi.astype(bf),
                "xTx": cT(x[b]),
                "cyT": np.ascontiguousarray(cy[b, rows].T).astype(bf),
                "yT": np.ascontiguousarray(y[b, rows].T).astype(f32),
                **w_shared,
            }
        )
    return in_maps


def run(inputs, trace=False, **kw):
    from concourse.bass_utils import run_bass_kernel_spmd

    nc = get_nc()
    in_maps = make_in_maps(inputs)
    res = run_bass_kernel_spmd(nc, in_maps, core_ids=list(range(N_CORES)), trace=trace, **kw)
    out = np.empty((B, S, D), np.float32)
    for c in range(N_CORES):
        b, half = divmod(c, 2)
        out[b, half * T : (half + 1) * T, :] = res.results[c]["y3T"].T
    return out, res


def kernel(**inputs):
    out, _ = run(inputs)
    return out
